# revision 27
# baseline (speedup 1.0000x reference)
"""CapsuleLayer dynamic-routing kernel for 8 Trainium2 NeuronCores (v7).

Problem: x [64,2048,16], route_weights [32,2048,16,32] ->
  3-iteration routing -> out [32,64,1,1,32] (fp32).

Sharding: capsules (C=32) split 4-per-core across 8 cores; x replicated.

v7 design (fp16 datapath, fp32 accumulation):
  - wt ([(c,o),(r,j)]) and xt2 ([(j,r),b]) resident in SBUF as fp16
    (chunk-loaded so dependent compute starts on first-arrival);
    w2cat ([(j,r),(c,o)]) streamed per use-phase, x2n ([h,(c,b32),(r,j)])
    streamed per boundary half.
  - Phase A: psA[b,co] += xt2_k^T @ w2cat_k  (stationary xt2, no transpose).
  - V-matmul: block-diagonal stationary OD[h][(c,o),(c,b32)] (built on-chip
    from out), dense K=128 matmuls with N=512 fp16 resident weights.
  - delta: DVE fp16 multiply (2x mode) + in-place pair-add tree over j.
  - per-half pipeline: half h's softmax/exp/transposes overlap half h+1's
    V-phase.
  - s-matmul: col-packed (tile_position=(0,32c)) 4-capsule matmuls
    accumulating into one psS[(c,o),b] PSUM tile; 1/Z folded into squash.
"""
import os
import numpy as np

C, B, R, CIN, OUT = 32, 64, 2048, 16, 32
NCORES = 8
CLOC = C // NCORES          # 4 capsules per core
RJ = R * CIN                # 32768
NK = RJ // 128              # 256 chunks of 128 along (j,r) / (r,j)
NG = 16                     # rj chunks of 2048 along (r,j)

_CACHE = {}


def _build_program():
    from contextlib import ExitStack
    import concourse.bass as bass
    import concourse.bacc as bacc
    import concourse.tile as tile
    from concourse import mybir

    f32 = mybir.dt.float32
    f16 = mybir.dt.float16
    AL = mybir.AluOpType
    AF = mybir.ActivationFunctionType
    AX = mybir.AxisListType

    nc = bacc.Bacc(None, target_bir_lowering=False,
                   detect_race_conditions=not bool(int(os.environ.get("CAPS_NO_RACE", "0"))))
    n_loops = int(os.environ.get("CAPS_LOOPS", "1"))

    # ---- DRAM I/O ----
    w2r = nc.dram_tensor("w2r", [128, NK, 128], f16, kind="ExternalInput")   # [rj%128, k, (c,o)]
    xt2r = nc.dram_tensor("xt2r", [128, NK, B], f16, kind="ExternalInput")   # [rj%128, k, b]
    wt = nc.dram_tensor("wt", [128, RJ], f16, kind="ExternalInput")          # [(c,o), (r,j)]
    x2n = nc.dram_tensor("x2n", [2, 128, RJ], f16, kind="ExternalInput")     # [h, (c,b32), (r,j)]
    ident = nc.dram_tensor("ident", [128, 128], f32, kind="ExternalInput")
    out3 = nc.dram_tensor("out3", [B, 128], f32, kind="ExternalOutput")      # [b, (c,o)]

    with tile.TileContext(nc) as tc, ExitStack() as ctx:
        const = ctx.enter_context(tc.tile_pool(name="const", bufs=1))
        small = ctx.enter_context(tc.tile_pool(name="small", bufs=3))
        w2s_p = ctx.enter_context(tc.tile_pool(name="w2s", bufs=3))
        x2k_p = ctx.enter_context(tc.tile_pool(name="x2k", bufs=3))
        vs_p = ctx.enter_context(tc.tile_pool(name="vs", bufs=3))
        xe_p = ctx.enter_context(tc.tile_pool(name="xe", bufs=2))
        eP_p = ctx.enter_context(tc.tile_pool(name="eP", bufs=2))
        od_p = ctx.enter_context(tc.tile_pool(name="od", bufs=2))
        psV_p = ctx.enter_context(tc.tile_pool(name="psV", bufs=2, space="PSUM"))
        psacc_p = ctx.enter_context(tc.tile_pool(name="psacc", bufs=1, space="PSUM"))
        psT_p = ctx.enter_context(tc.tile_pool(name="psT", bufs=2, space="PSUM"))

        idn = const.tile([128, 128], f32, tag="ident", name="idn")
        nc.sync.dma_start(out=idn, in_=ident[:])
        idn16 = const.tile([128, 128], f16, tag="ident16", name="idn16")
        nc.vector.tensor_copy(out=idn16, in_=idn)
        z128 = const.tile([128, 128], f16, tag="z128", name="z128")
        nc.vector.tensor_scalar_mul(out=z128, in0=idn16, scalar1=0.0)

        # resident fp16 tensors, chunk-loaded on the scalar DMA queue so
        # dependent compute can start as soon as the first chunks land
        wt_sb = [const.tile([128, 2048], f16, tag=f"wt{g}", name=f"wt_sb{g}")
                 for g in range(NG)]
        for g in range(NG):
            nc.scalar.dma_start(out=wt_sb[g], in_=wt[:, 2048 * g:2048 * (g + 1)])
        NCH = 8
        KC = NK // NCH
        xt2_sb = [const.tile([128, KC, B], f16, tag=f"xt2sb{i}", name=f"xt2_sb{i}")
                  for i in range(NCH)]
        for i in range(NCH):
            nc.scalar.dma_start(out=xt2_sb[i], in_=xt2r[:, KC * i:KC * (i + 1), :])

        def xt2k(k):
            return xt2_sb[k // KC][:, k % KC, :]

        # logits per b-half [(c,b32)=128, r=2048] fp32
        lP = [const.tile([128, R], f32, tag=f"l{h}", name=f"lP{h}") for h in range(2)]
        # transposed unnormalized probs [r%128, c4, rb, b] fp16
        p2T = const.tile([128, CLOC, R // 128, B], f16, tag="p2T", name="p2T")

        def squash(u_bT, rz=None, scale_pow=1.0):
            """u_bT [64,(4c,32o)] f32. If rz given ([64,4] f32 per-(b,c)
            scale), squash(u*rz); else squash(u*scale_pow)."""
            sq = small.tile([B, 128], f32, tag="sq", name="sq")
            n2 = small.tile([B, CLOC], f32, tag="n2", name="n2")
            if rz is None:
                nc.vector.scalar_tensor_tensor(
                    out=sq, in0=u_bT, scalar=float(scale_pow * scale_pow),
                    in1=u_bT, op0=AL.mult, op1=AL.mult)
                nc.vector.tensor_reduce(
                    out=n2, in_=sq[:].rearrange("b (c o) -> b c o", c=CLOC),
                    axis=AX.X, op=AL.add)
            else:
                nc.vector.tensor_mul(out=sq, in0=u_bT, in1=u_bT)
                q2 = small.tile([B, CLOC], f32, tag="q2", name="q2")
                nc.vector.tensor_reduce(
                    out=q2, in_=sq[:].rearrange("b (c o) -> b c o", c=CLOC),
                    axis=AX.X, op=AL.add)
                rz2 = small.tile([B, CLOC], f32, tag="rz2", name="rz2")
                nc.vector.tensor_mul(out=rz2, in0=rz, in1=rz)
                nc.vector.tensor_mul(out=n2, in0=q2, in1=rz2)
            rt = small.tile([B, CLOC], f32, tag="rt", name="rt")
            nc.scalar.activation(out=rt, in_=n2, func=AF.Sqrt)
            dn = small.tile([B, CLOC], f32, tag="dn", name="dn")
            nc.vector.tensor_scalar_add(out=dn, in0=n2, scalar1=1.0)
            rc = small.tile([B, CLOC], f32, tag="rc", name="rc")
            nc.vector.reciprocal(out=rc, in_=dn)
            f = small.tile([B, CLOC], f32, tag="f", name="f")
            nc.vector.tensor_mul(out=f, in0=rt, in1=rc)
            f2 = small.tile([B, CLOC], f32, tag="f2", name="f2")
            if rz is None:
                nc.vector.tensor_scalar_mul(out=f2, in0=f, scalar1=float(scale_pow))
            else:
                nc.vector.tensor_mul(out=f2, in0=f, in1=rz)
            o_i = small.tile([B, 128], f32, tag="oi", name="oi")
            f2b = bass.AP(tensor=f2[:].tensor, offset=f2[:].offset,
                          ap=[f2[:].ap[0], f2[:].ap[1], [0, OUT]])
            nc.vector.tensor_tensor(
                out=o_i[:].rearrange("b (c o) -> b c o", c=CLOC),
                in0=u_bT[:].rearrange("b (c o) -> b c o", c=CLOC),
                in1=f2b, op=AL.mult)
            psOT = psT_p.tile([128, B], f32, tag="psT", name="psOT")
            nc.tensor.transpose(psOT, o_i, idn[0:B, 0:B])
            oT = small.tile([128, B], f16, tag="oT", name="oT")
            nc.scalar.copy(out=oT, in_=psOT)
            return o_i, oT

        for _loop in range(n_loops):
            # ---------- Phase A: s1 = (1/R) sum_(j,r) x W ----------
            psA = psacc_p.tile([B, 128], f32, tag="psA", name="psA")
            for kg in range(NK // 16):
                w2s = w2s_p.tile([128, 16, 128], f16, tag="w2s", name="w2s")
                nc.sync.dma_start(out=w2s, in_=w2r[:, 16 * kg:16 * (kg + 1), :])
                for kk in range(16):
                    k = 16 * kg + kk
                    nc.tensor.matmul(psA, xt2k(k), w2s[:, kk, :],
                                     start=(k == 0), stop=(k == NK - 1))
            uA = small.tile([B, 128], f32, tag="uA", name="uA")
            nc.scalar.copy(out=uA, in_=psA)
            out_i, outT = squash(uA, scale_pow=1.0 / R)

            # ---------- Two routing boundaries ----------
            for it in (1, 2):
                # --- block-diag stationaries OD[h] from outT ---
                ODs = []
                for h in range(2):
                    OD = od_p.tile([128, 128], f16, tag=f"OD{h}", name=f"OD{h}")
                    nc.vector.tensor_copy(out=OD, in_=z128)
                    for c4 in range(CLOC):
                        nc.vector.tensor_copy(
                            out=OD[32 * c4:32 * (c4 + 1), 32 * c4:32 * (c4 + 1)],
                            in_=outT[32 * c4:32 * (c4 + 1), 32 * h:32 * (h + 1)])
                    ODs.append(OD)

                # --- V + delta + softmax, one batch-half at a time so the
                # softmax/transposes of half h overlap half h+1's V-phase ---
                zq = small.tile([B, CLOC], f32, tag="zq", name="zq")
                for h in range(2):
                    for g in range(NG):
                        x2k = x2k_p.tile([128, 2048], f16, tag="x2k", name="x2k")
                        nc.sync.dma_start(
                            out=x2k, in_=x2n[h, :, 2048 * g:2048 * (g + 1)])
                        vs = vs_p.tile([128, 2048], f16, tag="vs", name="vs")
                        for u in range(2):
                            psV = psV_p.tile([128, 1024], f32, tag="psV", name="psV")
                            for t in range(2):
                                nc.tensor.matmul(
                                    psV[:, 512 * t:512 * (t + 1)],
                                    ODs[h],
                                    wt_sb[g][:, 1024 * u + 512 * t:
                                             1024 * u + 512 * (t + 1)],
                                    start=True, stop=True)
                            nc.scalar.copy(out=vs[:, 1024 * u:1024 * (u + 1)],
                                           in_=psV)
                        nc.vector.tensor_mul(out=vs, in0=vs, in1=x2k)
                        # in-place j-sum tree: 16 -> 8 -> 4 -> 2 (2x fp16)
                        vr = vs[:].rearrange("p (r j) -> p r j", j=CIN)
                        nc.vector.tensor_add(out=vr[:, :, 0:8], in0=vr[:, :, 0:8],
                                             in1=vr[:, :, 8:16])
                        nc.vector.tensor_add(out=vr[:, :, 0:4], in0=vr[:, :, 0:4],
                                             in1=vr[:, :, 4:8])
                        nc.vector.tensor_add(out=vr[:, :, 0:2], in0=vr[:, :, 0:2],
                                             in1=vr[:, :, 2:4])
                        if it == 1:
                            nc.vector.tensor_reduce(
                                out=lP[h][:, 128 * g:128 * (g + 1)],
                                in_=vr[:, :, 0:2], axis=AX.X, op=AL.add)
                        else:
                            dtmp = small.tile([128, 128], f32, tag="dtmp", name="dtmp")
                            nc.vector.tensor_reduce(out=dtmp, in_=vr[:, :, 0:2],
                                                    axis=AX.X, op=AL.add)
                            nc.vector.tensor_add(
                                out=lP[h][:, 128 * g:128 * (g + 1)],
                                in0=lP[h][:, 128 * g:128 * (g + 1)], in1=dtmp)

                    # softmax pieces for this half (unnormalized e + Z)
                    m = small.tile([128, 1], f32, tag="m", name="m")
                    nc.vector.tensor_reduce(out=m, in_=lP[h], axis=AX.X, op=AL.max)
                    mneg = small.tile([128, 1], f32, tag="mneg", name="mneg")
                    nc.vector.tensor_scalar_mul(out=mneg, in0=m, scalar1=-1.0)
                    eP = eP_p.tile([128, R], f16, tag="eP", name="eP")
                    Z = small.tile([128, 1], f32, tag="Z", name="Z")
                    nc.scalar.activation(out=eP, in_=lP[h], func=AF.Exp,
                                         bias=mneg[:, 0:1], scale=1.0, accum_out=Z)
                    for c4 in range(CLOC):
                        nc.sync.dma_start(
                            out=zq[32 * h:32 * (h + 1), c4:c4 + 1],
                            in_=Z[32 * c4:32 * (c4 + 1), 0:1])
                    for rb in range(R // 128):
                        psT2 = psT_p.tile([128, 128], f16, tag="psT", name="psT2")
                        nc.tensor.transpose(
                            psT2, eP[:, 128 * rb:128 * (rb + 1)], idn16)
                        nc.scalar.copy(
                            out=p2T[:, :, rb, 32 * h:32 * (h + 1)],
                            in_=psT2[:].rearrange("p (c bh) -> p c bh", c=CLOC))
                rzq = small.tile([B, CLOC], f32, tag="rzq", name="rzq")
                nc.vector.reciprocal(out=rzq, in_=zq)

                # --- xe + s matmuls (w2cat streamed; group kg == j) ---
                psS = psacc_p.tile([128, B], f32, tag="psS", name="psS")
                for j in range(CIN):
                    w2s = w2s_p.tile([128, 16, 128], f16, tag="w2s", name="w2s")
                    nc.sync.dma_start(out=w2s, in_=w2r[:, 16 * j:16 * (j + 1), :])
                    xes = []
                    for c4 in range(CLOC):
                        xe = xe_p.tile([128, R // 128, B], f16, tag=f"xe{c4}",
                                       name=f"xe{c4}")
                        nc.vector.tensor_mul(
                            out=xe,
                            in0=xt2_sb[j // 2][:, 16 * (j % 2):16 * (j % 2 + 1), :],
                            in1=p2T[:, c4, :, :])
                        xes.append(xe)
                    for rb in range(R // 128):
                        k = 16 * j + rb
                        for c4 in range(CLOC):
                            nc.tensor.matmul(
                                psS[32 * c4:32 * (c4 + 1), :],
                                w2s[:, rb, 32 * c4:32 * (c4 + 1)],
                                xes[c4][:, rb, :],
                                start=(k == 0), stop=(k == NK - 1),
                                tile_position=(0, 32 * c4))
                # psS [(c,o), b] -> [b, (c,o)]
                sSt = small.tile([128, B], f32, tag="sSt", name="sSt")
                nc.scalar.copy(out=sSt, in_=psS)
                psSb = psT_p.tile([B, 128], f32, tag="psT", name="psSb")
                nc.tensor.transpose(psSb, sSt, idn)
                sSb = small.tile([B, 128], f32, tag="sSb", name="sSb")
                nc.scalar.copy(out=sSb, in_=psSb)
                out_i, outT = squash(sSb, rz=rzq)

            nc.sync.dma_start(out=out3[:], in_=out_i)

    nc.finalize()
    return nc


def _get_program():
    if "nc" not in _CACHE:
        _CACHE["nc"] = _build_program()
    return _CACHE["nc"]


def make_in_maps(x, route_weights):
    x = np.ascontiguousarray(x, dtype=np.float32)
    W = np.ascontiguousarray(route_weights, dtype=np.float32)
    # [(j,r), b] -> [rj%128, k, b]
    xt2 = x.transpose(2, 1, 0).reshape(RJ, B)
    xt2r = np.ascontiguousarray(
        xt2.reshape(NK, 128, B).transpose(1, 0, 2)).astype(np.float16)
    xnat = x.reshape(B, RJ)                                     # [b, (r,j)]
    # [h, (c,b32), (r,j)]: 4 capsule-replicas of each 32-batch half
    x2n = np.ascontiguousarray(
        np.stack([np.tile(xnat[0:32], (CLOC, 1)),
                  np.tile(xnat[32:64], (CLOC, 1))])).astype(np.float16)
    ident = np.eye(128, dtype=np.float32)
    in_maps = []
    for core in range(NCORES):
        wc = W[CLOC * core:CLOC * (core + 1)]                   # [4,R,J,O]
        # [(j,r),(c,o)] -> [rj%128, k, (c,o)]
        w2cat = wc.transpose(2, 1, 0, 3).reshape(RJ, CLOC * OUT)
        w2r = np.ascontiguousarray(
            w2cat.reshape(NK, 128, CLOC * OUT).transpose(1, 0, 2)).astype(np.float16)
        # [(c,o), (r,j)]
        wtc = np.ascontiguousarray(
            wc.transpose(0, 3, 1, 2).reshape(CLOC * OUT, RJ)).astype(np.float16)
        m = {"w2r": w2r, "xt2r": xt2r, "wt": wtc, "x2n": x2n, "ident": ident}
        in_maps.append(m)
    return in_maps


def kernel(x, route_weights):
    from concourse.bass_utils import run_bass_kernel_spmd

    in_maps = make_in_maps(x, route_weights)
    nc = _get_program()
    res = run_bass_kernel_spmd(nc, in_maps, core_ids=list(range(NCORES)))
    if os.environ.get("CAPS_RESULT_STASH"):
        _CACHE["last_result"] = res

    out = np.empty((C, B, 1, 1, OUT), dtype=np.float32)
    for core in range(NCORES):
        o = res.results[core]["out3"].reshape(B, CLOC, OUT).transpose(1, 0, 2)
        out[CLOC * core:CLOC * (core + 1), :, 0, 0, :] = o
    return out


# revision 28
# speedup vs baseline: 1.0306x; 1.0306x over previous
"""CapsuleLayer dynamic-routing kernel for 8 Trainium2 NeuronCores (v7).

Problem: x [64,2048,16], route_weights [32,2048,16,32] ->
  3-iteration routing -> out [32,64,1,1,32] (fp32).

Sharding: capsules (C=32) split 4-per-core across 8 cores; x replicated.

v7 design (fp16 datapath, fp32 accumulation):
  - wt ([(c,o),(r,j)]) and xt2 ([(j,r),b]) resident in SBUF as fp16
    (chunk-loaded so dependent compute starts on first-arrival);
    w2cat ([(j,r),(c,o)]) streamed per use-phase, x2n ([h,(c,b32),(r,j)])
    streamed per boundary half.
  - Phase A: psA[b,co] += xt2_k^T @ w2cat_k  (stationary xt2, no transpose).
  - V-matmul: block-diagonal stationary OD[h][(c,o),(c,b32)] (built on-chip
    from out), dense K=128 matmuls with N=512 fp16 resident weights.
  - delta: DVE fp16 multiply (2x mode) + in-place pair-add tree over j.
  - per-half pipeline: half h's softmax/exp/transposes overlap half h+1's
    V-phase.
  - s-matmul: col-packed (tile_position=(0,32c)) 4-capsule matmuls
    accumulating into one psS[(c,o),b] PSUM tile; 1/Z folded into squash.
"""
import os
import numpy as np

C, B, R, CIN, OUT = 32, 64, 2048, 16, 32
NCORES = 8
CLOC = C // NCORES          # 4 capsules per core
RJ = R * CIN                # 32768
NK = RJ // 128              # 256 chunks of 128 along (j,r) / (r,j)
NG = 16                     # rj chunks of 2048 along (r,j)

_CACHE = {}


def _build_program():
    from contextlib import ExitStack
    import concourse.bass as bass
    import concourse.bacc as bacc
    import concourse.tile as tile
    from concourse import mybir

    f32 = mybir.dt.float32
    f16 = mybir.dt.float16
    AL = mybir.AluOpType
    AF = mybir.ActivationFunctionType
    AX = mybir.AxisListType

    nc = bacc.Bacc(None, target_bir_lowering=False,
                   detect_race_conditions=not bool(int(os.environ.get("CAPS_NO_RACE", "0"))))
    n_loops = int(os.environ.get("CAPS_LOOPS", "1"))

    # ---- DRAM I/O ----
    w2r = nc.dram_tensor("w2r", [128, NK, 128], f16, kind="ExternalInput")   # [rj%128, k, (c,o)]
    xt2r = nc.dram_tensor("xt2r", [128, NK, B], f16, kind="ExternalInput")   # [rj%128, k, b]
    wt = nc.dram_tensor("wt", [128, RJ], f16, kind="ExternalInput")          # [(c,o), (r,j)]
    x2n = nc.dram_tensor("x2n", [2, 128, RJ], f16, kind="ExternalInput")     # [h, (c,b32), (r,j)]
    ident = nc.dram_tensor("ident", [128, 128], f32, kind="ExternalInput")
    out3 = nc.dram_tensor("out3", [B, 128], f32, kind="ExternalOutput")      # [b, (c,o)]

    with tile.TileContext(nc) as tc, ExitStack() as ctx:
        const = ctx.enter_context(tc.tile_pool(name="const", bufs=1))
        small = ctx.enter_context(tc.tile_pool(name="small", bufs=3))
        w2s_p = ctx.enter_context(tc.tile_pool(name="w2s", bufs=3))
        x2k_p = ctx.enter_context(tc.tile_pool(name="x2k", bufs=3))
        vs_p = ctx.enter_context(tc.tile_pool(name="vs", bufs=3))
        xe_p = ctx.enter_context(tc.tile_pool(name="xe", bufs=2))
        eP_p = ctx.enter_context(tc.tile_pool(name="eP", bufs=2))
        od_p = ctx.enter_context(tc.tile_pool(name="od", bufs=2))
        psV_p = ctx.enter_context(tc.tile_pool(name="psV", bufs=2, space="PSUM"))
        psacc_p = ctx.enter_context(tc.tile_pool(name="psacc", bufs=1, space="PSUM"))
        psT_p = ctx.enter_context(tc.tile_pool(name="psT", bufs=2, space="PSUM"))

        idn = const.tile([128, 128], f32, tag="ident", name="idn")
        nc.sync.dma_start(out=idn, in_=ident[:])
        idn16 = const.tile([128, 128], f16, tag="ident16", name="idn16")
        nc.vector.tensor_copy(out=idn16, in_=idn)
        z128 = const.tile([128, 128], f16, tag="z128", name="z128")
        nc.vector.tensor_scalar_mul(out=z128, in0=idn16, scalar1=0.0)

        # resident fp16 tensors, chunk-loaded on the scalar DMA queue so
        # dependent compute can start as soon as the first chunks land
        wt_sb = [const.tile([128, 2048], f16, tag=f"wt{g}", name=f"wt_sb{g}")
                 for g in range(NG)]
        NCH = 8
        KC = NK // NCH
        xt2_sb = [const.tile([128, KC, B], f16, tag=f"xt2sb{i}", name=f"xt2_sb{i}")
                  for i in range(NCH)]
        # xt2 first: phase A consumes it immediately; wt is only needed
        # once the first boundary's V-phase starts
        for i in range(NCH):
            nc.scalar.dma_start(out=xt2_sb[i], in_=xt2r[:, KC * i:KC * (i + 1), :])
        for g in range(NG):
            nc.scalar.dma_start(out=wt_sb[g], in_=wt[:, 2048 * g:2048 * (g + 1)])

        def xt2k(k):
            return xt2_sb[k // KC][:, k % KC, :]

        # logits per b-half [(c,b32)=128, r=2048] fp32
        lP = [const.tile([128, R], f32, tag=f"l{h}", name=f"lP{h}") for h in range(2)]
        # transposed unnormalized probs [r%128, c4, rb, b] fp16
        p2T = const.tile([128, CLOC, R // 128, B], f16, tag="p2T", name="p2T")

        def squash(u_bT, rz=None, scale_pow=1.0):
            """u_bT [64,(4c,32o)] f32. If rz given ([64,4] f32 per-(b,c)
            scale), squash(u*rz); else squash(u*scale_pow)."""
            sq = small.tile([B, 128], f32, tag="sq", name="sq")
            n2 = small.tile([B, CLOC], f32, tag="n2", name="n2")
            if rz is None:
                nc.vector.scalar_tensor_tensor(
                    out=sq, in0=u_bT, scalar=float(scale_pow * scale_pow),
                    in1=u_bT, op0=AL.mult, op1=AL.mult)
                nc.vector.tensor_reduce(
                    out=n2, in_=sq[:].rearrange("b (c o) -> b c o", c=CLOC),
                    axis=AX.X, op=AL.add)
            else:
                nc.vector.tensor_mul(out=sq, in0=u_bT, in1=u_bT)
                q2 = small.tile([B, CLOC], f32, tag="q2", name="q2")
                nc.vector.tensor_reduce(
                    out=q2, in_=sq[:].rearrange("b (c o) -> b c o", c=CLOC),
                    axis=AX.X, op=AL.add)
                rz2 = small.tile([B, CLOC], f32, tag="rz2", name="rz2")
                nc.vector.tensor_mul(out=rz2, in0=rz, in1=rz)
                nc.vector.tensor_mul(out=n2, in0=q2, in1=rz2)
            rt = small.tile([B, CLOC], f32, tag="rt", name="rt")
            nc.scalar.activation(out=rt, in_=n2, func=AF.Sqrt)
            dn = small.tile([B, CLOC], f32, tag="dn", name="dn")
            nc.vector.tensor_scalar_add(out=dn, in0=n2, scalar1=1.0)
            rc = small.tile([B, CLOC], f32, tag="rc", name="rc")
            nc.vector.reciprocal(out=rc, in_=dn)
            f = small.tile([B, CLOC], f32, tag="f", name="f")
            nc.vector.tensor_mul(out=f, in0=rt, in1=rc)
            f2 = small.tile([B, CLOC], f32, tag="f2", name="f2")
            if rz is None:
                nc.vector.tensor_scalar_mul(out=f2, in0=f, scalar1=float(scale_pow))
            else:
                nc.vector.tensor_mul(out=f2, in0=f, in1=rz)
            o_i = small.tile([B, 128], f32, tag="oi", name="oi")
            f2b = bass.AP(tensor=f2[:].tensor, offset=f2[:].offset,
                          ap=[f2[:].ap[0], f2[:].ap[1], [0, OUT]])
            nc.vector.tensor_tensor(
                out=o_i[:].rearrange("b (c o) -> b c o", c=CLOC),
                in0=u_bT[:].rearrange("b (c o) -> b c o", c=CLOC),
                in1=f2b, op=AL.mult)
            psOT = psT_p.tile([128, B], f32, tag="psT", name="psOT")
            nc.tensor.transpose(psOT, o_i, idn[0:B, 0:B])
            oT = small.tile([128, B], f16, tag="oT", name="oT")
            nc.scalar.copy(out=oT, in_=psOT)
            return o_i, oT

        for _loop in range(n_loops):
            # ---------- Phase A: s1 = (1/R) sum_(j,r) x W ----------
            psA = psacc_p.tile([B, 128], f32, tag="psA", name="psA")
            for kg in range(NK // 16):
                w2s = w2s_p.tile([128, 16, 128], f16, tag="w2s", name="w2s")
                nc.sync.dma_start(out=w2s, in_=w2r[:, 16 * kg:16 * (kg + 1), :])
                for kk in range(16):
                    k = 16 * kg + kk
                    nc.tensor.matmul(psA, xt2k(k), w2s[:, kk, :],
                                     start=(k == 0), stop=(k == NK - 1))
            uA = small.tile([B, 128], f32, tag="uA", name="uA")
            nc.scalar.copy(out=uA, in_=psA)
            out_i, outT = squash(uA, scale_pow=1.0 / R)

            # ---------- Two routing boundaries ----------
            for it in (1, 2):
                # --- block-diag stationaries OD[h] from outT ---
                ODs = []
                for h in range(2):
                    OD = od_p.tile([128, 128], f16, tag=f"OD{h}", name=f"OD{h}")
                    nc.vector.tensor_copy(out=OD, in_=z128)
                    for c4 in range(CLOC):
                        nc.vector.tensor_copy(
                            out=OD[32 * c4:32 * (c4 + 1), 32 * c4:32 * (c4 + 1)],
                            in_=outT[32 * c4:32 * (c4 + 1), 32 * h:32 * (h + 1)])
                    ODs.append(OD)

                # --- V + delta + softmax, one batch-half at a time so the
                # softmax/transposes of half h overlap half h+1's V-phase ---
                zq = small.tile([B, CLOC], f32, tag="zq", name="zq")
                for h in range(2):
                    for g in range(NG):
                        x2k = x2k_p.tile([128, 2048], f16, tag="x2k", name="x2k")
                        nc.sync.dma_start(
                            out=x2k, in_=x2n[h, :, 2048 * g:2048 * (g + 1)])
                        vs = vs_p.tile([128, 2048], f16, tag="vs", name="vs")
                        for u in range(2):
                            psV = psV_p.tile([128, 1024], f32, tag="psV", name="psV")
                            for t in range(2):
                                nc.tensor.matmul(
                                    psV[:, 512 * t:512 * (t + 1)],
                                    ODs[h],
                                    wt_sb[g][:, 1024 * u + 512 * t:
                                             1024 * u + 512 * (t + 1)],
                                    start=True, stop=True)
                            nc.scalar.copy(out=vs[:, 1024 * u:1024 * (u + 1)],
                                           in_=psV)
                        nc.vector.tensor_mul(out=vs, in0=vs, in1=x2k)
                        # in-place j-sum tree: 16 -> 8 -> 4 -> 2 (2x fp16)
                        vr = vs[:].rearrange("p (r j) -> p r j", j=CIN)
                        nc.vector.tensor_add(out=vr[:, :, 0:8], in0=vr[:, :, 0:8],
                                             in1=vr[:, :, 8:16])
                        nc.vector.tensor_add(out=vr[:, :, 0:4], in0=vr[:, :, 0:4],
                                             in1=vr[:, :, 4:8])
                        nc.vector.tensor_add(out=vr[:, :, 0:2], in0=vr[:, :, 0:2],
                                             in1=vr[:, :, 2:4])
                        if it == 1:
                            nc.vector.tensor_reduce(
                                out=lP[h][:, 128 * g:128 * (g + 1)],
                                in_=vr[:, :, 0:2], axis=AX.X, op=AL.add)
                        else:
                            dtmp = small.tile([128, 128], f32, tag="dtmp", name="dtmp")
                            nc.vector.tensor_reduce(out=dtmp, in_=vr[:, :, 0:2],
                                                    axis=AX.X, op=AL.add)
                            nc.vector.tensor_add(
                                out=lP[h][:, 128 * g:128 * (g + 1)],
                                in0=lP[h][:, 128 * g:128 * (g + 1)], in1=dtmp)

                    # softmax pieces for this half (unnormalized e + Z)
                    m = small.tile([128, 1], f32, tag="m", name="m")
                    nc.vector.tensor_reduce(out=m, in_=lP[h], axis=AX.X, op=AL.max)
                    mneg = small.tile([128, 1], f32, tag="mneg", name="mneg")
                    nc.vector.tensor_scalar_mul(out=mneg, in0=m, scalar1=-1.0)
                    eP = eP_p.tile([128, R], f16, tag="eP", name="eP")
                    Z = small.tile([128, 1], f32, tag="Z", name="Z")
                    nc.scalar.activation(out=eP, in_=lP[h], func=AF.Exp,
                                         bias=mneg[:, 0:1], scale=1.0, accum_out=Z)
                    for c4 in range(CLOC):
                        nc.sync.dma_start(
                            out=zq[32 * h:32 * (h + 1), c4:c4 + 1],
                            in_=Z[32 * c4:32 * (c4 + 1), 0:1])
                    for rb in range(R // 128):
                        psT2 = psT_p.tile([128, 128], f16, tag="psT", name="psT2")
                        nc.tensor.transpose(
                            psT2, eP[:, 128 * rb:128 * (rb + 1)], idn16)
                        nc.scalar.copy(
                            out=p2T[:, :, rb, 32 * h:32 * (h + 1)],
                            in_=psT2[:].rearrange("p (c bh) -> p c bh", c=CLOC))
                rzq = small.tile([B, CLOC], f32, tag="rzq", name="rzq")
                nc.vector.reciprocal(out=rzq, in_=zq)

                # --- xe + s matmuls (w2cat streamed; group kg == j) ---
                psS = psacc_p.tile([128, B], f32, tag="psS", name="psS")
                for j in range(CIN):
                    w2s = w2s_p.tile([128, 16, 128], f16, tag="w2s", name="w2s")
                    nc.sync.dma_start(out=w2s, in_=w2r[:, 16 * j:16 * (j + 1), :])
                    xes = []
                    for c4 in range(CLOC):
                        xe = xe_p.tile([128, R // 128, B], f16, tag=f"xe{c4}",
                                       name=f"xe{c4}")
                        nc.vector.tensor_mul(
                            out=xe,
                            in0=xt2_sb[j // 2][:, 16 * (j % 2):16 * (j % 2 + 1), :],
                            in1=p2T[:, c4, :, :])
                        xes.append(xe)
                    for rb in range(R // 128):
                        k = 16 * j + rb
                        for c4 in range(CLOC):
                            nc.tensor.matmul(
                                psS[32 * c4:32 * (c4 + 1), :],
                                w2s[:, rb, 32 * c4:32 * (c4 + 1)],
                                xes[c4][:, rb, :],
                                start=(k == 0), stop=(k == NK - 1),
                                tile_position=(0, 32 * c4))
                # psS [(c,o), b] -> [b, (c,o)]
                sSt = small.tile([128, B], f32, tag="sSt", name="sSt")
                nc.scalar.copy(out=sSt, in_=psS)
                psSb = psT_p.tile([B, 128], f32, tag="psT", name="psSb")
                nc.tensor.transpose(psSb, sSt, idn)
                sSb = small.tile([B, 128], f32, tag="sSb", name="sSb")
                nc.scalar.copy(out=sSb, in_=psSb)
                out_i, outT = squash(sSb, rz=rzq)

            nc.sync.dma_start(out=out3[:], in_=out_i)

    nc.finalize()
    return nc


def _get_program():
    if "nc" not in _CACHE:
        _CACHE["nc"] = _build_program()
    return _CACHE["nc"]


def make_in_maps(x, route_weights):
    x = np.ascontiguousarray(x, dtype=np.float32)
    W = np.ascontiguousarray(route_weights, dtype=np.float32)
    # [(j,r), b] -> [rj%128, k, b]
    xt2 = x.transpose(2, 1, 0).reshape(RJ, B)
    xt2r = np.ascontiguousarray(
        xt2.reshape(NK, 128, B).transpose(1, 0, 2)).astype(np.float16)
    xnat = x.reshape(B, RJ)                                     # [b, (r,j)]
    # [h, (c,b32), (r,j)]: 4 capsule-replicas of each 32-batch half
    x2n = np.ascontiguousarray(
        np.stack([np.tile(xnat[0:32], (CLOC, 1)),
                  np.tile(xnat[32:64], (CLOC, 1))])).astype(np.float16)
    ident = np.eye(128, dtype=np.float32)
    in_maps = []
    for core in range(NCORES):
        wc = W[CLOC * core:CLOC * (core + 1)]                   # [4,R,J,O]
        # [(j,r),(c,o)] -> [rj%128, k, (c,o)]
        w2cat = wc.transpose(2, 1, 0, 3).reshape(RJ, CLOC * OUT)
        w2r = np.ascontiguousarray(
            w2cat.reshape(NK, 128, CLOC * OUT).transpose(1, 0, 2)).astype(np.float16)
        # [(c,o), (r,j)]
        wtc = np.ascontiguousarray(
            wc.transpose(0, 3, 1, 2).reshape(CLOC * OUT, RJ)).astype(np.float16)
        m = {"w2r": w2r, "xt2r": xt2r, "wt": wtc, "x2n": x2n, "ident": ident}
        in_maps.append(m)
    return in_maps


def kernel(x, route_weights):
    from concourse.bass_utils import run_bass_kernel_spmd

    in_maps = make_in_maps(x, route_weights)
    nc = _get_program()
    res = run_bass_kernel_spmd(nc, in_maps, core_ids=list(range(NCORES)))
    if os.environ.get("CAPS_RESULT_STASH"):
        _CACHE["last_result"] = res

    out = np.empty((C, B, 1, 1, OUT), dtype=np.float32)
    for core in range(NCORES):
        o = res.results[core]["out3"].reshape(B, CLOC, OUT).transpose(1, 0, 2)
        out[CLOC * core:CLOC * (core + 1), :, 0, 0, :] = o
    return out


# revision 31
# speedup vs baseline: 1.1033x; 1.0706x over previous
"""CapsuleLayer dynamic-routing kernel for 8 Trainium2 NeuronCores (v7).

Problem: x [64,2048,16], route_weights [32,2048,16,32] ->
  3-iteration routing -> out [32,64,1,1,32] (fp32).

Sharding: capsules (C=32) split 4-per-core across 8 cores; x replicated.

v7 design (fp16 datapath, fp32 accumulation):
  - wt ([(c,o),(r,j)]) and xt2 ([(j,r),b]) resident in SBUF as fp16
    (chunk-loaded so dependent compute starts on first-arrival);
    w2cat ([(j,r),(c,o)]) streamed per use-phase, x2n ([h,(c,b32),(r,j)])
    streamed per boundary half.
  - Phase A: psA[b,co] += xt2_k^T @ w2cat_k  (stationary xt2, no transpose).
  - V-matmul: block-diagonal stationary OD[h][(c,o),(c,b32)] (built on-chip
    from out), dense K=128 matmuls with N=512 fp16 resident weights.
  - delta: DVE fp16 multiply (2x mode) + in-place pair-add tree over j.
  - per-half pipeline: half h's softmax/exp/transposes overlap half h+1's
    V-phase.
  - s-matmul: col-packed (tile_position=(0,32c)) 4-capsule matmuls
    accumulating into one psS[(c,o),b] PSUM tile; 1/Z folded into squash.
"""
import os
import numpy as np

C, B, R, CIN, OUT = 32, 64, 2048, 16, 32
NCORES = 8
CLOC = C // NCORES          # 4 capsules per core
RJ = R * CIN                # 32768
NK = RJ // 128              # 256 chunks of 128 along (j,r) / (r,j)
NG = 16                     # rj chunks of 2048 along (r,j)

_CACHE = {}


def _build_program():
    from contextlib import ExitStack
    import concourse.bass as bass
    import concourse.bacc as bacc
    import concourse.tile as tile
    from concourse import mybir

    f32 = mybir.dt.float32
    f16 = mybir.dt.float16
    AL = mybir.AluOpType
    AF = mybir.ActivationFunctionType
    AX = mybir.AxisListType

    nc = bacc.Bacc(None, target_bir_lowering=False,
                   detect_race_conditions=not bool(int(os.environ.get("CAPS_NO_RACE", "0"))))
    n_loops = int(os.environ.get("CAPS_LOOPS", "1"))

    # ---- DRAM I/O ----
    w2r = nc.dram_tensor("w2r", [128, NK, 128], f16, kind="ExternalInput")   # [rj%128, k, (c,o)]
    xt2r = nc.dram_tensor("xt2r", [128, NK, B], f16, kind="ExternalInput")   # [rj%128, k, b]
    wt = nc.dram_tensor("wt", [128, RJ], f16, kind="ExternalInput")          # [(c,o), (r,j)]
    x2n = nc.dram_tensor("x2n", [2, 128, RJ], f16, kind="ExternalInput")     # [h, (c,b32), (r,j)]
    ident = nc.dram_tensor("ident", [128, 128], f32, kind="ExternalInput")
    out3 = nc.dram_tensor("out3", [B, 128], f32, kind="ExternalOutput")      # [b, (c,o)]

    with tile.TileContext(nc) as tc, ExitStack() as ctx:
        const = ctx.enter_context(tc.tile_pool(name="const", bufs=1))
        small = ctx.enter_context(tc.tile_pool(name="small", bufs=3))
        w2s_p = ctx.enter_context(tc.tile_pool(name="w2s", bufs=3))
        x2k_p = ctx.enter_context(tc.tile_pool(name="x2k", bufs=3))
        vs_p = ctx.enter_context(tc.tile_pool(name="vs", bufs=3))
        xe_p = ctx.enter_context(tc.tile_pool(name="xe", bufs=2))
        eP_p = ctx.enter_context(tc.tile_pool(name="eP", bufs=2))
        od_p = ctx.enter_context(tc.tile_pool(name="od", bufs=2))
        psV_p = ctx.enter_context(tc.tile_pool(name="psV", bufs=2, space="PSUM"))
        psacc_p = ctx.enter_context(tc.tile_pool(name="psacc", bufs=1, space="PSUM"))
        psT_p = ctx.enter_context(tc.tile_pool(name="psT", bufs=2, space="PSUM"))

        idn = const.tile([128, 128], f32, tag="ident", name="idn")
        nc.sync.dma_start(out=idn, in_=ident[:])
        idn16 = const.tile([128, 128], f16, tag="ident16", name="idn16")
        nc.vector.tensor_copy(out=idn16, in_=idn)
        z128 = const.tile([128, 128], f16, tag="z128", name="z128")
        nc.vector.tensor_scalar_mul(out=z128, in0=idn16, scalar1=0.0)

        # resident fp16 tensors, chunk-loaded on the scalar DMA queue so
        # dependent compute can start as soon as the first chunks land
        wt_sb = [const.tile([128, 2048], f16, tag=f"wt{g}", name=f"wt_sb{g}")
                 for g in range(NG)]
        NCH = 8
        KC = NK // NCH
        xt2_sb = [const.tile([128, KC, B], f16, tag=f"xt2sb{i}", name=f"xt2_sb{i}")
                  for i in range(NCH)]
        # xt2 first: phase A consumes it immediately; wt chunks are
        # enqueued just-in-time inside the first boundary's V-loop so they
        # don't steal HBM bandwidth from phase A's w2cat stream
        for i in range(NCH):
            nc.scalar.dma_start(out=xt2_sb[i], in_=xt2r[:, KC * i:KC * (i + 1), :])

        def xt2k(k):
            return xt2_sb[k // KC][:, k % KC, :]

        # logits per b-half [(c,b32)=128, r=2048] fp32
        lP = [const.tile([128, R], f32, tag=f"l{h}", name=f"lP{h}") for h in range(2)]
        # transposed unnormalized probs [r%128, c4, rb, b] fp16
        p2T = const.tile([128, CLOC, R // 128, B], f16, tag="p2T", name="p2T")

        def squash(u_bT, rz=None, scale_pow=1.0):
            """u_bT [64,(4c,32o)] f32. If rz given ([64,4] f32 per-(b,c)
            scale), squash(u*rz); else squash(u*scale_pow)."""
            sq = small.tile([B, 128], f32, tag="sq", name="sq")
            n2 = small.tile([B, CLOC], f32, tag="n2", name="n2")
            if rz is None:
                nc.vector.scalar_tensor_tensor(
                    out=sq, in0=u_bT, scalar=float(scale_pow * scale_pow),
                    in1=u_bT, op0=AL.mult, op1=AL.mult)
                nc.vector.tensor_reduce(
                    out=n2, in_=sq[:].rearrange("b (c o) -> b c o", c=CLOC),
                    axis=AX.X, op=AL.add)
            else:
                nc.vector.tensor_mul(out=sq, in0=u_bT, in1=u_bT)
                q2 = small.tile([B, CLOC], f32, tag="q2", name="q2")
                nc.vector.tensor_reduce(
                    out=q2, in_=sq[:].rearrange("b (c o) -> b c o", c=CLOC),
                    axis=AX.X, op=AL.add)
                rz2 = small.tile([B, CLOC], f32, tag="rz2", name="rz2")
                nc.vector.tensor_mul(out=rz2, in0=rz, in1=rz)
                nc.vector.tensor_mul(out=n2, in0=q2, in1=rz2)
            rt = small.tile([B, CLOC], f32, tag="rt", name="rt")
            nc.scalar.activation(out=rt, in_=n2, func=AF.Sqrt)
            dn = small.tile([B, CLOC], f32, tag="dn", name="dn")
            nc.vector.tensor_scalar_add(out=dn, in0=n2, scalar1=1.0)
            rc = small.tile([B, CLOC], f32, tag="rc", name="rc")
            nc.vector.reciprocal(out=rc, in_=dn)
            f = small.tile([B, CLOC], f32, tag="f", name="f")
            nc.vector.tensor_mul(out=f, in0=rt, in1=rc)
            f2 = small.tile([B, CLOC], f32, tag="f2", name="f2")
            if rz is None:
                nc.vector.tensor_scalar_mul(out=f2, in0=f, scalar1=float(scale_pow))
            else:
                nc.vector.tensor_mul(out=f2, in0=f, in1=rz)
            o_i = small.tile([B, 128], f32, tag="oi", name="oi")
            f2b = bass.AP(tensor=f2[:].tensor, offset=f2[:].offset,
                          ap=[f2[:].ap[0], f2[:].ap[1], [0, OUT]])
            nc.vector.tensor_tensor(
                out=o_i[:].rearrange("b (c o) -> b c o", c=CLOC),
                in0=u_bT[:].rearrange("b (c o) -> b c o", c=CLOC),
                in1=f2b, op=AL.mult)
            psOT = psT_p.tile([128, B], f32, tag="psT", name="psOT")
            nc.tensor.transpose(psOT, o_i, idn[0:B, 0:B])
            oT = small.tile([128, B], f16, tag="oT", name="oT")
            nc.scalar.copy(out=oT, in_=psOT)
            return o_i, oT

        for _loop in range(n_loops):
            # ---------- Phase A: s1 = (1/R) sum_(j,r) x W ----------
            psA = psacc_p.tile([B, 128], f32, tag="psA", name="psA")
            for kg in range(NK // 16):
                w2s = w2s_p.tile([128, 16, 128], f16, tag="w2s", name="w2s")
                nc.sync.dma_start(out=w2s, in_=w2r[:, 16 * kg:16 * (kg + 1), :])
                for kk in range(16):
                    k = 16 * kg + kk
                    nc.tensor.matmul(psA, xt2k(k), w2s[:, kk, :],
                                     start=(k == 0), stop=(k == NK - 1))
            uA = small.tile([B, 128], f32, tag="uA", name="uA")
            nc.scalar.copy(out=uA, in_=psA)
            out_i, outT = squash(uA, scale_pow=1.0 / R)

            # ---------- Two routing boundaries ----------
            for it in (1, 2):
                # --- block-diag stationaries OD[h] from outT ---
                ODs = []
                for h in range(2):
                    OD = od_p.tile([128, 128], f16, tag=f"OD{h}", name=f"OD{h}")
                    nc.vector.tensor_copy(out=OD, in_=z128)
                    for c4 in range(CLOC):
                        nc.vector.tensor_copy(
                            out=OD[32 * c4:32 * (c4 + 1), 32 * c4:32 * (c4 + 1)],
                            in_=outT[32 * c4:32 * (c4 + 1), 32 * h:32 * (h + 1)])
                    ODs.append(OD)

                # --- V + delta + softmax, one batch-half at a time so the
                # softmax/transposes of half h overlap half h+1's V-phase ---
                zq = small.tile([B, CLOC], f32, tag="zq", name="zq")
                for h in range(2):
                    for g in range(NG):
                        if _loop == 0 and it == 1 and h == 0:
                            nc.sync.dma_start(
                                out=wt_sb[g], in_=wt[:, 2048 * g:2048 * (g + 1)])
                        x2k = x2k_p.tile([128, 2048], f16, tag="x2k", name="x2k")
                        nc.sync.dma_start(
                            out=x2k, in_=x2n[h, :, 2048 * g:2048 * (g + 1)])
                        vs = vs_p.tile([128, 2048], f16, tag="vs", name="vs")
                        for u in range(2):
                            psV = psV_p.tile([128, 1024], f32, tag="psV", name="psV")
                            for t in range(2):
                                nc.tensor.matmul(
                                    psV[:, 512 * t:512 * (t + 1)],
                                    ODs[h],
                                    wt_sb[g][:, 1024 * u + 512 * t:
                                             1024 * u + 512 * (t + 1)],
                                    start=True, stop=True)
                            nc.scalar.copy(out=vs[:, 1024 * u:1024 * (u + 1)],
                                           in_=psV)
                        nc.vector.tensor_mul(out=vs, in0=vs, in1=x2k)
                        # in-place j-sum tree: 16 -> 8 -> 4 -> 2 (2x fp16)
                        vr = vs[:].rearrange("p (r j) -> p r j", j=CIN)
                        nc.vector.tensor_add(out=vr[:, :, 0:8], in0=vr[:, :, 0:8],
                                             in1=vr[:, :, 8:16])
                        nc.vector.tensor_add(out=vr[:, :, 0:4], in0=vr[:, :, 0:4],
                                             in1=vr[:, :, 4:8])
                        nc.vector.tensor_add(out=vr[:, :, 0:2], in0=vr[:, :, 0:2],
                                             in1=vr[:, :, 2:4])
                        if it == 1:
                            nc.vector.tensor_reduce(
                                out=lP[h][:, 128 * g:128 * (g + 1)],
                                in_=vr[:, :, 0:2], axis=AX.X, op=AL.add)
                        else:
                            dtmp = small.tile([128, 128], f32, tag="dtmp", name="dtmp")
                            nc.vector.tensor_reduce(out=dtmp, in_=vr[:, :, 0:2],
                                                    axis=AX.X, op=AL.add)
                            nc.vector.tensor_add(
                                out=lP[h][:, 128 * g:128 * (g + 1)],
                                in0=lP[h][:, 128 * g:128 * (g + 1)], in1=dtmp)

                    # softmax pieces for this half (unnormalized e + Z)
                    m = small.tile([128, 1], f32, tag="m", name="m")
                    nc.vector.tensor_reduce(out=m, in_=lP[h], axis=AX.X, op=AL.max)
                    mneg = small.tile([128, 1], f32, tag="mneg", name="mneg")
                    nc.vector.tensor_scalar_mul(out=mneg, in0=m, scalar1=-1.0)
                    eP = eP_p.tile([128, R], f16, tag="eP", name="eP")
                    Z = small.tile([128, 1], f32, tag="Z", name="Z")
                    nc.scalar.activation(out=eP, in_=lP[h], func=AF.Exp,
                                         bias=mneg[:, 0:1], scale=1.0, accum_out=Z)
                    for c4 in range(CLOC):
                        nc.sync.dma_start(
                            out=zq[32 * h:32 * (h + 1), c4:c4 + 1],
                            in_=Z[32 * c4:32 * (c4 + 1), 0:1])
                    for rb in range(R // 128):
                        psT2 = psT_p.tile([128, 128], f16, tag="psT", name="psT2")
                        nc.tensor.transpose(
                            psT2, eP[:, 128 * rb:128 * (rb + 1)], idn16)
                        nc.scalar.copy(
                            out=p2T[:, :, rb, 32 * h:32 * (h + 1)],
                            in_=psT2[:].rearrange("p (c bh) -> p c bh", c=CLOC))
                rzq = small.tile([B, CLOC], f32, tag="rzq", name="rzq")
                nc.vector.reciprocal(out=rzq, in_=zq)

                # --- xe + s matmuls (w2cat streamed; group kg == j) ---
                psS = psacc_p.tile([128, B], f32, tag="psS", name="psS")
                for j in range(CIN):
                    w2s = w2s_p.tile([128, 16, 128], f16, tag="w2s", name="w2s")
                    nc.sync.dma_start(out=w2s, in_=w2r[:, 16 * j:16 * (j + 1), :])
                    xes = []
                    for c4 in range(CLOC):
                        xe = xe_p.tile([128, R // 128, B], f16, tag=f"xe{c4}",
                                       name=f"xe{c4}")
                        nc.vector.tensor_mul(
                            out=xe,
                            in0=xt2_sb[j // 2][:, 16 * (j % 2):16 * (j % 2 + 1), :],
                            in1=p2T[:, c4, :, :])
                        xes.append(xe)
                    for rb in range(R // 128):
                        k = 16 * j + rb
                        for c4 in range(CLOC):
                            nc.tensor.matmul(
                                psS[32 * c4:32 * (c4 + 1), :],
                                w2s[:, rb, 32 * c4:32 * (c4 + 1)],
                                xes[c4][:, rb, :],
                                start=(k == 0), stop=(k == NK - 1),
                                tile_position=(0, 32 * c4))
                # psS [(c,o), b] -> [b, (c,o)]
                sSt = small.tile([128, B], f32, tag="sSt", name="sSt")
                nc.scalar.copy(out=sSt, in_=psS)
                psSb = psT_p.tile([B, 128], f32, tag="psT", name="psSb")
                nc.tensor.transpose(psSb, sSt, idn)
                sSb = small.tile([B, 128], f32, tag="sSb", name="sSb")
                nc.scalar.copy(out=sSb, in_=psSb)
                out_i, outT = squash(sSb, rz=rzq)

            nc.sync.dma_start(out=out3[:], in_=out_i)

    nc.finalize()
    return nc


def _get_program():
    if "nc" not in _CACHE:
        _CACHE["nc"] = _build_program()
    return _CACHE["nc"]


def make_in_maps(x, route_weights):
    x = np.ascontiguousarray(x, dtype=np.float32)
    W = np.ascontiguousarray(route_weights, dtype=np.float32)
    # [(j,r), b] -> [rj%128, k, b]
    xt2 = x.transpose(2, 1, 0).reshape(RJ, B)
    xt2r = np.ascontiguousarray(
        xt2.reshape(NK, 128, B).transpose(1, 0, 2)).astype(np.float16)
    xnat = x.reshape(B, RJ)                                     # [b, (r,j)]
    # [h, (c,b32), (r,j)]: 4 capsule-replicas of each 32-batch half
    x2n = np.ascontiguousarray(
        np.stack([np.tile(xnat[0:32], (CLOC, 1)),
                  np.tile(xnat[32:64], (CLOC, 1))])).astype(np.float16)
    ident = np.eye(128, dtype=np.float32)
    in_maps = []
    for core in range(NCORES):
        wc = W[CLOC * core:CLOC * (core + 1)]                   # [4,R,J,O]
        # [(j,r),(c,o)] -> [rj%128, k, (c,o)]
        w2cat = wc.transpose(2, 1, 0, 3).reshape(RJ, CLOC * OUT)
        w2r = np.ascontiguousarray(
            w2cat.reshape(NK, 128, CLOC * OUT).transpose(1, 0, 2)).astype(np.float16)
        # [(c,o), (r,j)]
        wtc = np.ascontiguousarray(
            wc.transpose(0, 3, 1, 2).reshape(CLOC * OUT, RJ)).astype(np.float16)
        m = {"w2r": w2r, "xt2r": xt2r, "wt": wtc, "x2n": x2n, "ident": ident}
        in_maps.append(m)
    return in_maps


def kernel(x, route_weights):
    from concourse.bass_utils import run_bass_kernel_spmd

    in_maps = make_in_maps(x, route_weights)
    nc = _get_program()
    res = run_bass_kernel_spmd(nc, in_maps, core_ids=list(range(NCORES)))
    if os.environ.get("CAPS_RESULT_STASH"):
        _CACHE["last_result"] = res

    out = np.empty((C, B, 1, 1, OUT), dtype=np.float32)
    for core in range(NCORES):
        o = res.results[core]["out3"].reshape(B, CLOC, OUT).transpose(1, 0, 2)
        out[CLOC * core:CLOC * (core + 1), :, 0, 0, :] = o
    return out


# revision 34
# speedup vs baseline: 1.1162x; 1.0117x over previous
"""CapsuleLayer dynamic-routing kernel for 8 Trainium2 NeuronCores (v7).

Problem: x [64,2048,16], route_weights [32,2048,16,32] ->
  3-iteration routing -> out [32,64,1,1,32] (fp32).

Sharding: capsules (C=32) split 4-per-core across 8 cores; x replicated.

v7 design (fp16 datapath, fp32 accumulation):
  - wt ([(c,o),(r,j)]) and xt2 ([(j,r),b]) resident in SBUF as fp16
    (chunk-loaded so dependent compute starts on first-arrival);
    w2cat ([(j,r),(c,o)]) streamed per use-phase, x2n ([h,(c,b32),(r,j)])
    streamed per boundary half.
  - Phase A: psA[b,co] += xt2_k^T @ w2cat_k  (stationary xt2, no transpose).
  - V-matmul: block-diagonal stationary OD[h][(c,o),(c,b32)] (built on-chip
    from out), dense K=128 matmuls with N=512 fp16 resident weights.
  - delta: DVE fp16 multiply (2x mode) + in-place pair-add tree over j.
  - per-half pipeline: half h's softmax/exp/transposes overlap half h+1's
    V-phase.
  - s-matmul: col-packed (tile_position=(0,32c)) 4-capsule matmuls
    accumulating into one psS[(c,o),b] PSUM tile; 1/Z folded into squash.
"""
import os
import numpy as np

C, B, R, CIN, OUT = 32, 64, 2048, 16, 32
NCORES = 8
CLOC = C // NCORES          # 4 capsules per core
RJ = R * CIN                # 32768
NK = RJ // 128              # 256 chunks of 128 along (j,r) / (r,j)
NG = 16                     # rj chunks of 2048 along (r,j)

_CACHE = {}


def _build_program():
    from contextlib import ExitStack
    import concourse.bass as bass
    import concourse.bacc as bacc
    import concourse.tile as tile
    from concourse import mybir

    f32 = mybir.dt.float32
    f16 = mybir.dt.float16
    AL = mybir.AluOpType
    AF = mybir.ActivationFunctionType
    AX = mybir.AxisListType

    nc = bacc.Bacc(None, target_bir_lowering=False,
                   detect_race_conditions=not bool(int(os.environ.get("CAPS_NO_RACE", "0"))))
    n_loops = int(os.environ.get("CAPS_LOOPS", "1"))
    gp_vs = int(os.environ.get("CAPS_GP_VS", "1"))   # of every 4 (g,h) mults on gpsimd

    # ---- DRAM I/O ----
    w2r = nc.dram_tensor("w2r", [128, NK, 128], f16, kind="ExternalInput")   # [rj%128, k, (c,o)]
    xt2r = nc.dram_tensor("xt2r", [128, NK, B], f16, kind="ExternalInput")   # [rj%128, k, b]
    wt = nc.dram_tensor("wt", [128, RJ], f16, kind="ExternalInput")          # [(c,o), (r,j)]
    x2n = nc.dram_tensor("x2n", [2, 128, RJ], f16, kind="ExternalInput")     # [h, (c,b32), (r,j)]
    ident = nc.dram_tensor("ident", [128, 128], f32, kind="ExternalInput")
    out3 = nc.dram_tensor("out3", [B, 128], f32, kind="ExternalOutput")      # [b, (c,o)]

    with tile.TileContext(nc) as tc, ExitStack() as ctx:
        const = ctx.enter_context(tc.tile_pool(name="const", bufs=1))
        small = ctx.enter_context(tc.tile_pool(name="small", bufs=3))
        w2s_p = ctx.enter_context(tc.tile_pool(name="w2s", bufs=3))
        x2k_p = ctx.enter_context(tc.tile_pool(name="x2k", bufs=4))
        vs_p = ctx.enter_context(tc.tile_pool(name="vs", bufs=3))
        vsg_p = ctx.enter_context(tc.tile_pool(name="vsg", bufs=2))
        xe_p = ctx.enter_context(tc.tile_pool(name="xe", bufs=2))
        eP_p = ctx.enter_context(tc.tile_pool(name="eP", bufs=2))
        od_p = ctx.enter_context(tc.tile_pool(name="od", bufs=2))
        psV_p = ctx.enter_context(tc.tile_pool(name="psV", bufs=2, space="PSUM"))
        psacc_p = ctx.enter_context(tc.tile_pool(name="psacc", bufs=1, space="PSUM"))
        psT_p = ctx.enter_context(tc.tile_pool(name="psT", bufs=2, space="PSUM"))

        idn = const.tile([128, 128], f32, tag="ident", name="idn")
        nc.sync.dma_start(out=idn, in_=ident[:])
        idn16 = const.tile([128, 128], f16, tag="ident16", name="idn16")
        nc.vector.tensor_copy(out=idn16, in_=idn)
        z128 = const.tile([128, 128], f16, tag="z128", name="z128")
        nc.vector.tensor_scalar_mul(out=z128, in0=idn16, scalar1=0.0)

        # resident fp16 tensors, chunk-loaded on the scalar DMA queue so
        # dependent compute can start as soon as the first chunks land
        wt_sb = [const.tile([128, 2048], f16, tag=f"wt{g}", name=f"wt_sb{g}")
                 for g in range(NG)]
        NCH = 8
        KC = NK // NCH
        xt2_sb = [const.tile([128, KC, B], f16, tag=f"xt2sb{i}", name=f"xt2_sb{i}")
                  for i in range(NCH)]
        # xt2 first: phase A consumes it immediately; wt chunks are
        # enqueued just-in-time inside the first boundary's V-loop so they
        # don't steal HBM bandwidth from phase A's w2cat stream
        for i in range(NCH):
            nc.scalar.dma_start(out=xt2_sb[i], in_=xt2r[:, KC * i:KC * (i + 1), :])

        def xt2k(k):
            return xt2_sb[k // KC][:, k % KC, :]

        # logits per b-half [(c,b32)=128, r=2048] fp32
        lP = [const.tile([128, R], f32, tag=f"l{h}", name=f"lP{h}") for h in range(2)]
        # transposed unnormalized probs [r%128, c4, rb, b] fp16
        p2T = const.tile([128, CLOC, R // 128, B], f16, tag="p2T", name="p2T")

        def squash(u_bT, rz=None, scale_pow=1.0):
            """u_bT [64,(4c,32o)] f32. If rz given ([64,4] f32 per-(b,c)
            scale), squash(u*rz); else squash(u*scale_pow)."""
            sq = small.tile([B, 128], f32, tag="sq", name="sq")
            n2 = small.tile([B, CLOC], f32, tag="n2", name="n2")
            if rz is None:
                nc.vector.scalar_tensor_tensor(
                    out=sq, in0=u_bT, scalar=float(scale_pow * scale_pow),
                    in1=u_bT, op0=AL.mult, op1=AL.mult)
                nc.vector.tensor_reduce(
                    out=n2, in_=sq[:].rearrange("b (c o) -> b c o", c=CLOC),
                    axis=AX.X, op=AL.add)
            else:
                nc.vector.tensor_mul(out=sq, in0=u_bT, in1=u_bT)
                q2 = small.tile([B, CLOC], f32, tag="q2", name="q2")
                nc.vector.tensor_reduce(
                    out=q2, in_=sq[:].rearrange("b (c o) -> b c o", c=CLOC),
                    axis=AX.X, op=AL.add)
                rz2 = small.tile([B, CLOC], f32, tag="rz2", name="rz2")
                nc.vector.tensor_mul(out=rz2, in0=rz, in1=rz)
                nc.vector.tensor_mul(out=n2, in0=q2, in1=rz2)
            rt = small.tile([B, CLOC], f32, tag="rt", name="rt")
            nc.scalar.activation(out=rt, in_=n2, func=AF.Sqrt)
            dn = small.tile([B, CLOC], f32, tag="dn", name="dn")
            nc.vector.tensor_scalar_add(out=dn, in0=n2, scalar1=1.0)
            rc = small.tile([B, CLOC], f32, tag="rc", name="rc")
            nc.vector.reciprocal(out=rc, in_=dn)
            f = small.tile([B, CLOC], f32, tag="f", name="f")
            nc.vector.tensor_mul(out=f, in0=rt, in1=rc)
            f2 = small.tile([B, CLOC], f32, tag="f2", name="f2")
            if rz is None:
                nc.vector.tensor_scalar_mul(out=f2, in0=f, scalar1=float(scale_pow))
            else:
                nc.vector.tensor_mul(out=f2, in0=f, in1=rz)
            o_i = small.tile([B, 128], f32, tag="oi", name="oi")
            f2b = bass.AP(tensor=f2[:].tensor, offset=f2[:].offset,
                          ap=[f2[:].ap[0], f2[:].ap[1], [0, OUT]])
            nc.vector.tensor_tensor(
                out=o_i[:].rearrange("b (c o) -> b c o", c=CLOC),
                in0=u_bT[:].rearrange("b (c o) -> b c o", c=CLOC),
                in1=f2b, op=AL.mult)
            psOT = psT_p.tile([128, B], f32, tag="psT", name="psOT")
            nc.tensor.transpose(psOT, o_i, idn[0:B, 0:B])
            oT = small.tile([128, B], f16, tag="oT", name="oT")
            nc.scalar.copy(out=oT, in_=psOT)
            return o_i, oT

        for _loop in range(n_loops):
            # ---------- Phase A: s1 = (1/R) sum_(j,r) x W ----------
            psA = psacc_p.tile([B, 128], f32, tag="psA", name="psA")
            for kg in range(NK // 16):
                w2s = w2s_p.tile([128, 16, 128], f16, tag="w2s", name="w2s")
                nc.sync.dma_start(out=w2s, in_=w2r[:, 16 * kg:16 * (kg + 1), :])
                for kk in range(16):
                    k = 16 * kg + kk
                    nc.tensor.matmul(psA, xt2k(k), w2s[:, kk, :],
                                     start=(k == 0), stop=(k == NK - 1))
            uA = small.tile([B, 128], f32, tag="uA", name="uA")
            nc.scalar.copy(out=uA, in_=psA)
            out_i, outT = squash(uA, scale_pow=1.0 / R)

            # ---------- Two routing boundaries ----------
            for it in (1, 2):
                # --- block-diag stationaries OD[h] from outT ---
                ODs = []
                for h in range(2):
                    OD = od_p.tile([128, 128], f16, tag=f"OD{h}", name=f"OD{h}")
                    nc.vector.tensor_copy(out=OD, in_=z128)
                    for c4 in range(CLOC):
                        nc.vector.tensor_copy(
                            out=OD[32 * c4:32 * (c4 + 1), 32 * c4:32 * (c4 + 1)],
                            in_=outT[32 * c4:32 * (c4 + 1), 32 * h:32 * (h + 1)])
                    ODs.append(OD)

                # --- V + delta + softmax, one batch-half at a time so the
                # softmax/transposes of half h overlap half h+1's V-phase ---
                zq = small.tile([B, CLOC], f32, tag="zq", name="zq")

                def tree_and_store(vs, g, it, h):
                    # in-place j-sum tree 16->8->4->2 (2x fp16), strided
                    # final pair-add straight into the logits slice
                    vr = vs[:].rearrange("p (r j) -> p r j", j=CIN)
                    nc.vector.tensor_add(out=vr[:, :, 0:8], in0=vr[:, :, 0:8],
                                         in1=vr[:, :, 8:16])
                    nc.vector.tensor_add(out=vr[:, :, 0:4], in0=vr[:, :, 0:4],
                                         in1=vr[:, :, 4:8])
                    nc.vector.tensor_add(out=vr[:, :, 0:2], in0=vr[:, :, 0:2],
                                         in1=vr[:, :, 2:4])
                    a0 = vr[:, :, 0:1].rearrange("p r j -> p (r j)")
                    a1 = vr[:, :, 1:2].rearrange("p r j -> p (r j)")
                    lslice = lP[h][:, 128 * g:128 * (g + 1)]
                    if it == 1:
                        nc.vector.tensor_tensor(out=lslice, in0=a0, in1=a1,
                                                op=AL.add)
                    else:
                        dtmp = small.tile([128, 128], f32, tag="dtmp", name="dtmp")
                        nc.vector.tensor_tensor(out=dtmp, in0=a0, in1=a1,
                                                op=AL.add)
                        nc.vector.tensor_add(out=lslice, in0=lslice, in1=dtmp)

                for h in range(2):
                    pending = []
                    for g in range(NG):
                        if _loop == 0 and it == 1 and h == 0:
                            nc.sync.dma_start(
                                out=wt_sb[g], in_=wt[:, 2048 * g:2048 * (g + 1)])
                        on_gp = (g % 4) >= 4 - gp_vs
                        x2k = x2k_p.tile([128, 2048], f16, tag="x2k", name="x2k")
                        nc.sync.dma_start(
                            out=x2k, in_=x2n[h, :, 2048 * g:2048 * (g + 1)])
                        my_vs = vsg_p if on_gp else vs_p
                        vs = my_vs.tile([128, 2048], f16, tag="vs", name="vs")
                        for u in range(2):
                            psV = psV_p.tile([128, 1024], f32, tag="psV", name="psV")
                            for t in range(2):
                                nc.tensor.matmul(
                                    psV[:, 512 * t:512 * (t + 1)],
                                    ODs[h],
                                    wt_sb[g][:, 1024 * u + 512 * t:
                                             1024 * u + 512 * (t + 1)],
                                    start=True, stop=True)
                            nc.scalar.copy(out=vs[:, 1024 * u:1024 * (u + 1)],
                                           in_=psV)
                        if on_gp:
                            # gpsimd does the big multiply; its (DVE) tree is
                            # deferred so the DVE FIFO never waits on gpsimd
                            nc.gpsimd.tensor_mul(out=vs, in0=vs, in1=x2k)
                            pending.append((vs, g))
                        else:
                            nc.vector.tensor_mul(out=vs, in0=vs, in1=x2k)
                            tree_and_store(vs, g, it, h)
                            while pending and g - pending[0][1] >= 2:
                                pvs, pg = pending.pop(0)
                                tree_and_store(pvs, pg, it, h)
                    for pvs, pg in pending:
                        tree_and_store(pvs, pg, it, h)

                    # softmax pieces for this half (unnormalized e + Z)
                    m = small.tile([128, 1], f32, tag="m", name="m")
                    nc.vector.tensor_reduce(out=m, in_=lP[h], axis=AX.X, op=AL.max)
                    mneg = small.tile([128, 1], f32, tag="mneg", name="mneg")
                    nc.vector.tensor_scalar_mul(out=mneg, in0=m, scalar1=-1.0)
                    eP = eP_p.tile([128, R], f16, tag="eP", name="eP")
                    Z = small.tile([128, 1], f32, tag="Z", name="Z")
                    nc.scalar.activation(out=eP, in_=lP[h], func=AF.Exp,
                                         bias=mneg[:, 0:1], scale=1.0, accum_out=Z)
                    for c4 in range(CLOC):
                        nc.sync.dma_start(
                            out=zq[32 * h:32 * (h + 1), c4:c4 + 1],
                            in_=Z[32 * c4:32 * (c4 + 1), 0:1])
                    for rb in range(R // 128):
                        psT2 = psT_p.tile([128, 128], f16, tag="psT", name="psT2")
                        nc.tensor.transpose(
                            psT2, eP[:, 128 * rb:128 * (rb + 1)], idn16)
                        nc.scalar.copy(
                            out=p2T[:, :, rb, 32 * h:32 * (h + 1)],
                            in_=psT2[:].rearrange("p (c bh) -> p c bh", c=CLOC))
                rzq = small.tile([B, CLOC], f32, tag="rzq", name="rzq")
                nc.vector.reciprocal(out=rzq, in_=zq)

                # --- xe + s matmuls (w2cat streamed; group kg == j) ---
                psS = psacc_p.tile([128, B], f32, tag="psS", name="psS")
                for j in range(CIN):
                    w2s = w2s_p.tile([128, 16, 128], f16, tag="w2s", name="w2s")
                    nc.sync.dma_start(out=w2s, in_=w2r[:, 16 * j:16 * (j + 1), :])
                    xes = []
                    for c4 in range(CLOC):
                        xe = xe_p.tile([128, R // 128, B], f16, tag=f"xe{c4}",
                                       name=f"xe{c4}")
                        nc.vector.tensor_mul(
                            out=xe,
                            in0=xt2_sb[j // 2][:, 16 * (j % 2):16 * (j % 2 + 1), :],
                            in1=p2T[:, c4, :, :])
                        xes.append(xe)
                    for rb in range(R // 128):
                        k = 16 * j + rb
                        for c4 in range(CLOC):
                            nc.tensor.matmul(
                                psS[32 * c4:32 * (c4 + 1), :],
                                w2s[:, rb, 32 * c4:32 * (c4 + 1)],
                                xes[c4][:, rb, :],
                                start=(k == 0), stop=(k == NK - 1),
                                tile_position=(0, 32 * c4))
                # psS [(c,o), b] -> [b, (c,o)]
                sSt = small.tile([128, B], f32, tag="sSt", name="sSt")
                nc.scalar.copy(out=sSt, in_=psS)
                psSb = psT_p.tile([B, 128], f32, tag="psT", name="psSb")
                nc.tensor.transpose(psSb, sSt, idn)
                sSb = small.tile([B, 128], f32, tag="sSb", name="sSb")
                nc.scalar.copy(out=sSb, in_=psSb)
                out_i, outT = squash(sSb, rz=rzq)

            nc.sync.dma_start(out=out3[:], in_=out_i)

    nc.finalize()
    return nc


def _get_program():
    if "nc" not in _CACHE:
        _CACHE["nc"] = _build_program()
    return _CACHE["nc"]


def make_in_maps(x, route_weights):
    x = np.ascontiguousarray(x, dtype=np.float32)
    W = np.ascontiguousarray(route_weights, dtype=np.float32)
    # [(j,r), b] -> [rj%128, k, b]
    xt2 = x.transpose(2, 1, 0).reshape(RJ, B)
    xt2r = np.ascontiguousarray(
        xt2.reshape(NK, 128, B).transpose(1, 0, 2)).astype(np.float16)
    xnat = x.reshape(B, RJ)                                     # [b, (r,j)]
    # [h, (c,b32), (r,j)]: 4 capsule-replicas of each 32-batch half
    x2n = np.ascontiguousarray(
        np.stack([np.tile(xnat[0:32], (CLOC, 1)),
                  np.tile(xnat[32:64], (CLOC, 1))])).astype(np.float16)
    ident = np.eye(128, dtype=np.float32)
    in_maps = []
    for core in range(NCORES):
        wc = W[CLOC * core:CLOC * (core + 1)]                   # [4,R,J,O]
        # [(j,r),(c,o)] -> [rj%128, k, (c,o)]
        w2cat = wc.transpose(2, 1, 0, 3).reshape(RJ, CLOC * OUT)
        w2r = np.ascontiguousarray(
            w2cat.reshape(NK, 128, CLOC * OUT).transpose(1, 0, 2)).astype(np.float16)
        # [(c,o), (r,j)]
        wtc = np.ascontiguousarray(
            wc.transpose(0, 3, 1, 2).reshape(CLOC * OUT, RJ)).astype(np.float16)
        m = {"w2r": w2r, "xt2r": xt2r, "wt": wtc, "x2n": x2n, "ident": ident}
        in_maps.append(m)
    return in_maps


def kernel(x, route_weights):
    from concourse.bass_utils import run_bass_kernel_spmd

    in_maps = make_in_maps(x, route_weights)
    nc = _get_program()
    res = run_bass_kernel_spmd(nc, in_maps, core_ids=list(range(NCORES)))
    if os.environ.get("CAPS_RESULT_STASH"):
        _CACHE["last_result"] = res

    out = np.empty((C, B, 1, 1, OUT), dtype=np.float32)
    for core in range(NCORES):
        o = res.results[core]["out3"].reshape(B, CLOC, OUT).transpose(1, 0, 2)
        out[CLOC * core:CLOC * (core + 1), :, 0, 0, :] = o
    return out
